# revision 13
# baseline (speedup 1.0000x reference)
"""AttentionBlock kernel for Trainium2 (Bass/Tile), 8 NeuronCores.

Reference computation (B=4, C=256, H=W=64, Cqk=32, N=H*W=4096):
    q = Wq @ x + bq; k = Wk @ x + bk; v = Wv @ x + bv      (1x1 convs)
    energy[b,i,j] = sum_c q[b,c,i] k[b,c,j]
    attn = softmax(energy, axis=-1)
    out[b,c,i] = sum_j v[b,c,j] attn[b,i,j]
    result = gamma * out + x

Sharding: 8 cores = (batch b in 0..3) x (query-row half in 0..1).
Each core computes 2048 of the 4096 attention rows for one batch image;
the small conv weights are replicated.

Per-core layout choices:
  - energy is computed TRANSPOSED: energyT[j, i] with j on partitions.
    exp() is layout-agnostic; the softmax denominator s_i = sum_j exp()
    is obtained from a ones-column appended to vT in the attn@v matmul
    (outT[:, 256] = s_i), so no partition-axis reduction is ever needed.
  - No max-subtraction in softmax: |energy| <= ~45 for these scales, so
    exp() stays comfortably inside fp32 range; softmax ratios are exact.
  - The attn@v matmul produces outT[i, c] = sum_j expT[j,i] vT[j,c],
    normalized by gamma/s_i per partition, then PE-transposed back to
    [c, i] for the residual add with x.
  - PACK_E: the energy matmul has contraction Cqk=32, so 4 j-chunks run
    concurrently in the PE array as 32-row tiles (tile_position row
    packing). q/k are built 4x-replicated along partitions by tiling the
    projection weights host-side (zero extra device cost).
"""

import os

import numpy as np

B, C, H, W = 4, 256, 64, 64
CQK = 32
N = H * W                      # 4096
NCORES = 8
HALVES = 2                     # query-row halves per batch
NI = N // HALVES               # 2048 rows per core
P = 128                        # SBUF partitions
CC = C // P                    # 2 channel chunks
NJ = N // P                    # 32 key/value chunks
SW = 512                       # i-strip width
NSTRIP = NI // SW              # 4 strips per core
NT_K = N // SW                 # 8 k-proj tiles
NT_Q = NI // SW                # 4 q-proj tiles
CP = C + 4                     # vT width: 256 v-ch + ones col + pad (f32r %4)
G = 4                          # row-packing group size (128 / CQK)

# Defaults (HW-validated): attn@v matmul in float32r (PE 2 cyc/row vs 4
# for fp32; ~1e-4 scale-relative error), energy matmul in exact fp32 but
# row-packed 4-wide (K=32 << 128, so packing beats dtype tricks and stays
# exact), projections in exact fp32.
_ENERGY_DT = os.environ.get("KERNEL_ENERGY_DT", "float32")
_AV_DT = os.environ.get("KERNEL_AV_DT", "float32r")
_PACK_E = bool(int(os.environ.get("KERNEL_PACK_E", "1")))
# Benchmark-only: repeat the computation R times in a hardware loop so
# device time dominates the (slow) tunnel round-trip.
_REPEAT = int(os.environ.get("KERNEL_REPEAT", "1"))

_CACHE = {}
LAST_RESULT = None

QKP = P if _PACK_E else CQK    # partition height of q/k tiles


def _build_program():
    import contextlib

    import concourse.bacc as bacc
    import concourse.bass as bass
    import concourse.mybir as mybir
    import concourse.tile as tile
    from concourse.bass import ts
    from concourse.masks import make_identity

    f32 = mybir.dt.float32
    e_dt = getattr(mybir.dt, _ENERGY_DT)
    av_dt = getattr(mybir.dt, _AV_DT)
    AF = mybir.ActivationFunctionType

    nc = bacc.Bacc("TRN2", target_bir_lowering=False, debug=False)

    xb_d = nc.dram_tensor("xb", [C, N], f32, kind="ExternalInput")
    wqT_d = nc.dram_tensor("wqT", [C, QKP], f32, kind="ExternalInput")
    wkT_d = nc.dram_tensor("wkT", [C, QKP], f32, kind="ExternalInput")
    wvT_d = nc.dram_tensor("wvT", [C, CP], f32, kind="ExternalInput")
    bq_d = nc.dram_tensor("bq", [QKP], f32, kind="ExternalInput")
    bk_d = nc.dram_tensor("bk", [QKP], f32, kind="ExternalInput")
    bv_d = nc.dram_tensor("bv", [CP], f32, kind="ExternalInput")
    gam_d = nc.dram_tensor("gamma", [1], f32, kind="ExternalInput")
    out_d = nc.dram_tensor("out", [C, NI], f32, kind="ExternalOutput")

    with tile.TileContext(nc) as tc:
        with (
            tc.tile_pool(name="consts", bufs=1) as consts,
            tc.tile_pool(name="sb", bufs=1) as sb,
            tc.tile_pool(name="evac", bufs=3) as evac,
            tc.tile_pool(name="expp", bufs=2 if _PACK_E else 3) as expp,
            tc.tile_pool(name="psE", bufs=1 if _PACK_E else 2, space="PSUM") as psE,
            tc.tile_pool(name="psO", bufs=4, space="PSUM") as psO,
        ):
            ctx_psM = (
                contextlib.nullcontext()
                if _PACK_E
                else tc.tile_pool(name="psM", bufs=2, space="PSUM")
            )
            with ctx_psM as psM:
                # ---- constants / weights ----
                ident = consts.tile([P, P], f32)
                make_identity(nc, ident[:, :])

                wq_sb = consts.tile([P, CC, QKP], f32)
                nc.sync.dma_start(
                    out=wq_sb[:, :, :],
                    in_=wqT_d.ap().rearrange("(cc p) o -> p cc o", p=P),
                )
                wk_sb = consts.tile([P, CC, QKP], f32)
                nc.sync.dma_start(
                    out=wk_sb[:, :, :],
                    in_=wkT_d.ap().rearrange("(cc p) o -> p cc o", p=P),
                )
                wv_sb = consts.tile([P, CC, CP], f32)
                nc.sync.dma_start(
                    out=wv_sb[:, :, :],
                    in_=wvT_d.ap().rearrange("(cc p) c -> p cc c", p=P),
                )

                bq_sb = consts.tile([QKP, 1], f32)
                nc.gpsimd.dma_start(
                    out=bq_sb[:, :], in_=bass.AP(bq_d, 0, [[1, QKP], [1, 1]])
                )
                bk_sb = consts.tile([QKP, 1], f32)
                nc.gpsimd.dma_start(
                    out=bk_sb[:, :], in_=bass.AP(bk_d, 0, [[1, QKP], [1, 1]])
                )
                # bv broadcast along partitions (trailing 1.0 = ones column)
                bvb_sb = consts.tile([P, CP], f32)
                nc.gpsimd.dma_start(
                    out=bvb_sb[:, :], in_=bass.AP(bv_d, 0, [[0, P], [1, CP]])
                )
                gam_sb = consts.tile([P, 1], f32)
                nc.gpsimd.dma_start(
                    out=gam_sb[:, :], in_=bass.AP(gam_d, 0, [[0, P], [1, 1]])
                )

                rep = (
                    tc.For_i(0, _REPEAT, 1)
                    if _REPEAT > 1
                    else contextlib.nullcontext()
                )
                with rep:
                    # ---- activations ----
                    # x arrives column-rotated so this core's 2048 query
                    # columns are always cols 0:NI (attention is permutation-
                    # invariant over key/value positions, so rotating the key
                    # axis changes nothing). 4 DMAs to spread across queues.
                    xb_sb = sb.tile([P, CC, N], f32)
                    xb_src = xb_d.ap().rearrange("(cc p) n -> p cc n", p=P)
                    for d in range(4):
                        nc.sync.dma_start(
                            out=xb_sb[:, :, ts(d, N // 4)],
                            in_=xb_src[:, :, ts(d, N // 4)],
                        )
                    xq_sb = xb_sb[:, :, 0:NI]

                    q_sb = sb.tile([QKP, NI], e_dt)
                    k_sb = sb.tile([QKP, N], e_dt)
                    vt_sb = sb.tile([P, NJ, CP], av_dt)
                    out_sb = sb.tile([P, CC, NI], f32)

                    # ---- projections ----
                    # k = Wk @ xb + bk (PACK_E: 4x-replicated on partitions)
                    for t in range(NT_K):
                        ps = psO.tile([QKP, SW], f32, tag="po", name=f"psk{t}")
                        for cc in range(CC):
                            nc.tensor.matmul(
                                ps[:, :],
                                wk_sb[:, cc, :],
                                xb_sb[:, cc, ts(t, SW)],
                                start=(cc == 0),
                                stop=(cc == CC - 1),
                            )
                        nc.vector.tensor_scalar_add(
                            k_sb[:, ts(t, SW)], ps[:, :], bk_sb[:, :]
                        )
                    # q = Wq @ xq + bq
                    for t in range(NT_Q):
                        ps = psO.tile([QKP, SW], f32, tag="po", name=f"psq{t}")
                        for cc in range(CC):
                            nc.tensor.matmul(
                                ps[:, :],
                                wq_sb[:, cc, :],
                                xq_sb[:, cc, ts(t, SW)],
                                start=(cc == 0),
                                stop=(cc == CC - 1),
                            )
                        nc.vector.tensor_scalar_add(
                            q_sb[:, ts(t, SW)], ps[:, :], bq_sb[:, :]
                        )
                    # vT = (Wv @ xb + bv).T -> [4096, CP]; wvT's zero columns
                    # plus bv's trailing 1.0 produce the ones column that
                    # yields the softmax denominator in the attn@v matmul.
                    for j in range(NJ):
                        ps = psO.tile([P, CP], f32, tag="po", name=f"psv{j}")
                        for cc in range(CC):
                            nc.tensor.matmul(
                                ps[:, :],
                                xb_sb[:, cc, ts(j, P)],
                                wv_sb[:, cc, :],
                                start=(cc == 0),
                                stop=(cc == CC - 1),
                            )
                        nc.vector.tensor_add(vt_sb[:, j, :], ps[:, :], bvb_sb[:, :])

                    # ---- attention strips ----
                    for s in range(NSTRIP):
                        po = [
                            psO.tile([P, CP], f32, tag="po", name=f"po{s}_{u}")
                            for u in range(SW // P)
                        ]
                        if _PACK_E:
                            for t in range(NJ // G):
                                pe4 = psE.tile([P, G, SW], f32, tag="pe")
                                for g in range(G):
                                    j = G * t + g
                                    nc.tensor.matmul(
                                        pe4[:, g, :],
                                        k_sb[32 * g : 32 * (g + 1), ts(j, P)],
                                        q_sb[32 * g : 32 * (g + 1), ts(s, SW)],
                                        start=True,
                                        stop=True,
                                        tile_position=(32 * g, 0),
                                    )
                                ex4 = expp.tile([P, G, SW], av_dt, tag="ex")
                                nc.scalar.activation(
                                    ex4[:, :, :], pe4[:, :, :], AF.Exp
                                )
                                for g in range(G):
                                    j = G * t + g
                                    for u in range(SW // P):
                                        nc.tensor.matmul(
                                            po[u][:, :],
                                            ex4[:, g, ts(u, P)],
                                            vt_sb[:, j, :],
                                            start=(t == 0 and g == 0),
                                            stop=(
                                                t == NJ // G - 1 and g == G - 1
                                            ),
                                        )
                        else:
                            for j in range(NJ):
                                pe = psE.tile([P, SW], f32, tag="pe")
                                nc.tensor.matmul(
                                    pe[:, :],
                                    k_sb[:, ts(j, P)],
                                    q_sb[:, ts(s, SW)],
                                    start=True,
                                    stop=True,
                                )
                                ex = expp.tile([P, SW], av_dt, tag="ex")
                                nc.scalar.activation(ex[:, :], pe[:, :], AF.Exp)
                                for u in range(SW // P):
                                    nc.tensor.matmul(
                                        po[u][:, :],
                                        ex[:, ts(u, P)],
                                        vt_sb[:, j, :],
                                        start=(j == 0),
                                        stop=(j == NJ - 1),
                                    )
                        for u in range(SW // P):
                            i0 = s * SW + u * P
                            r = evac.tile([P, 1], f32, tag="r")
                            nc.vector.reciprocal(r[:, :], po[u][:, C : C + 1])
                            r2 = evac.tile([P, 1], f32, tag="r2")
                            nc.vector.tensor_scalar_mul(
                                r2[:, :], r[:, :], gam_sb[:, :]
                            )
                            osb = evac.tile([P, C], f32, tag="osb")
                            nc.vector.tensor_scalar_mul(
                                osb[:, :], po[u][:, 0:C], r2[:, :]
                            )
                            for ch in range(CC):
                                pool = psO if _PACK_E else psM
                                pt = pool.tile(
                                    [P, P],
                                    f32,
                                    tag="po" if _PACK_E else "ps",
                                    name=f"pt{s}_{u}_{ch}",
                                )
                                nc.tensor.transpose(
                                    pt[:, :], osb[:, ts(ch, P)], ident[:, :]
                                )
                                nc.vector.tensor_add(
                                    out_sb[:, ch, i0 : i0 + P],
                                    pt[:, :],
                                    xq_sb[:, ch, i0 : i0 + P],
                                )
                        nc.sync.dma_start(
                            out=out_d.ap().rearrange("(cc p) n -> p cc n", p=P)[
                                :, :, ts(s, SW)
                            ],
                            in_=out_sb[:, :, ts(s, SW)],
                        )

    nc.compile()
    return nc


def _host_prep(inputs):
    """Common host-side input preparation for all variants."""
    x = np.ascontiguousarray(np.asarray(inputs["x"], dtype=np.float32))
    Wq = np.asarray(inputs["Wq"], dtype=np.float32)
    Wk = np.asarray(inputs["Wk"], dtype=np.float32)
    Wv = np.asarray(inputs["Wv"], dtype=np.float32)
    bq = np.ascontiguousarray(np.asarray(inputs["bq"], dtype=np.float32))
    bk = np.ascontiguousarray(np.asarray(inputs["bk"], dtype=np.float32))
    bv = np.ascontiguousarray(np.asarray(inputs["bv"], dtype=np.float32))
    gamma = np.ascontiguousarray(np.asarray(inputs["gamma"], dtype=np.float32))

    xf = x.reshape(B, C, N)
    wqT = np.ascontiguousarray(Wq.T)
    wkT = np.ascontiguousarray(Wk.T)
    if _PACK_E:
        wqT = np.ascontiguousarray(np.tile(wqT, (1, G)))
        wkT = np.ascontiguousarray(np.tile(wkT, (1, G)))
        bq = np.ascontiguousarray(np.tile(bq, G))
        bk = np.ascontiguousarray(np.tile(bk, G))
    wvT = np.ascontiguousarray(
        np.concatenate([Wv.T, np.zeros((C, CP - C), np.float32)], axis=1)
    )
    bvp = np.concatenate(
        [bv, np.ones((1,), np.float32), np.zeros((CP - C - 1,), np.float32)]
    )

    in_maps = []
    for core in range(NCORES):
        b, half = divmod(core, HALVES)
        sl = slice(half * NI, (half + 1) * NI)
        in_maps.append(
            {
                "xb": np.ascontiguousarray(np.roll(xf[b], -half * NI, axis=1)),
                "wqT": wqT,
                "wkT": wkT,
                "wvT": wvT,
                "bq": bq,
                "bk": bk,
                "bv": bvp,
                "gamma": gamma,
            }
        )
    return in_maps


def kernel(**inputs):
    global LAST_RESULT
    from concourse.bass_utils import run_bass_kernel_spmd

    if "nc" not in _CACHE:
        _CACHE["nc"] = _build_program()
    nc = _CACHE["nc"]

    in_maps = _host_prep(inputs)

    trace = bool(os.environ.get("KERNEL_TRACE"))
    kwargs = {}
    if trace and os.environ.get("KERNEL_TRACE_ALL"):
        kwargs["trace_cores"] = list(range(NCORES))
        kwargs["stitch_traces"] = True
    res = run_bass_kernel_spmd(
        nc, in_maps, core_ids=list(range(NCORES)), trace=trace, **kwargs
    )
    LAST_RESULT = res

    out = np.empty((B, C, N), dtype=np.float32)
    for core in range(NCORES):
        b, half = divmod(core, HALVES)
        out[b][:, half * NI : (half + 1) * NI] = res.results[core]["out"]
    return out.reshape(B, C, H, W)
